# revision 15
# baseline (speedup 1.0000x reference)
"""AdaptiveGraphConv on 8 TRN2 NeuronCores (Bass/Tile).

Strategy: destination-sharded edge-parallel.  Edges are sorted by dst and
sharded by dst-node range (NLOC nodes/core), so each core owns the full
aggregation for its dst range and NO all-reduce of the [N, D] aggregate is
needed.  Node features h (and A = h@We1_top + be1) are computed node-parallel
and exchanged with a single AllGather of a bf16 [h|A] table; B = h@We1_bot is
only ever indexed by dst (always core-local, kept resident in SBUF).

Per-edge work uses dma_gather (SWDGE, 4 queues) for [h|A][src] + TensorE
matmuls: transpose-accumulate builds the edge-MLP pre-activation in PSUM, a
one-hot matmul does the segment scatter-add per 128-dst group.

Descriptor-count reduction: most edges are packed into COLUMN-ALIGNED tiles
(slot partition p <=> dst column p of the group), so their B-term is a single
matmul against the resident B shard - no per-edge B gather.  Per-column
overflow edges go to regular tiles with a (small) B gather.  dma_gather
indices are signed int16, so src gathers run against two table views
(rows [0,32768) and [32768,NPAD)); every region is split lo/hi by src id.
Tile regions per group, in processing order: [alo][ahi][ovlo][ovhi].
"""

import sys
import types

sys.path.insert(0, "/opt/trn_rl_repo")

import numpy as np
import ml_dtypes

import concourse.bass as bass
import concourse.bacc as bacc
import concourse.tile as tile
from concourse import mybir
from concourse.bass_utils import run_bass_kernel_spmd

BF16 = ml_dtypes.bfloat16
F32 = mybir.dt.float32
BF = mybir.dt.bfloat16
I16 = mybir.dt.int16

N_CORES = 8
D = 128
P = 128
SPLIT = 32768      # int16 index limit for dma_gather
CHUNK_G = 4        # groups per gather call batch
SCORE_BATCH = 4    # tiles per relu batch


def _install_ntff_hook():
    if "antenv.axon_hooks" in sys.modules:
        return
    try:
        from trn_agent_boot.trn_boot import _ntff_profile_via_ctypes

        hook = _ntff_profile_via_ctypes("/opt/axon/libaxon_pjrt.so")
    except Exception:
        hook = None
    mod = types.ModuleType("antenv.axon_hooks")
    mod.get_axon_ntff_profile_hook = lambda: hook
    mod.set_axon_ntff_profile_hook = lambda h: None
    sys.modules["antenv.axon_hooks"] = mod


# ----------------------------------------------------------------------------
# device program
# ----------------------------------------------------------------------------

def _build_program(NG, NLOC, NPAD, T_alo, T_ahi, T_ovlo, T_ovhi):
    T_lo = [a + o for a, o in zip(T_alo, T_ovlo)]
    T_hi = [a + o for a, o in zip(T_ahi, T_ovhi)]
    T_ov = [a + b for a, b in zip(T_ovlo, T_ovhi)]
    T_all = [a + b for a, b in zip(T_lo, T_hi)]
    NT = sum(T_all)
    NTlo = sum(T_lo)
    NThi = sum(T_hi)
    NTb = sum(T_ov)
    hi_base = min(SPLIT, NPAD)

    nc = bacc.Bacc(
        "TRN2", target_bir_lowering=False, debug=False, num_devices=N_CORES,
        num_swdge_queues=4,
    )

    def din(name, shape, dt):
        return nc.dram_tensor(name, list(shape), dt, kind="ExternalInput").ap()

    xT = din("xT", [P, NLOC], F32)
    W1 = din("W1", [P, D], F32)
    b1row = din("b1row", [1, D], F32)
    ones1f = din("ones1f", [1, D], F32)
    ones1b = din("ones1b", [1, D], BF)
    g1b = din("g1b", [P, D], F32)
    bt1b = din("bt1b", [P, D], F32)
    We1T = din("We1T", [P, D], BF)
    We1B = din("We1B", [P, D], BF)
    be1row = din("be1row", [1, D], BF)
    We2c = din("We2c", [P, 1], BF)
    be2c = din("be2c", [P, 1], F32)
    WgT = din("WgT", [P, D], BF)
    WgB = din("WgB", [P, D], BF)
    bgc = din("bgc", [P, 1], F32)
    g2b = din("g2b", [P, D], F32)
    bt2b = din("bt2b", [P, D], F32)
    idn = din("idn", [P, P], BF)
    iota = din("iota", [P, P], BF)
    ixlo = din("ixlo", [P, max(NTlo, 1) * 8], I16)
    ixhi = din("ixhi", [P, max(NThi, 1) * 8], I16)
    ixb = din("ixb", [P, max(NTb, 1) * 8], I16)
    colrel = din("colrel", [P, max(NT, 1)], F32)

    out = nc.dram_tensor("out", [NLOC, D], F32, kind="ExternalOutput").ap()

    chunks = []
    g0 = 0
    while g0 < NG:
        chunks.append((g0, min(g0 + CHUNK_G, NG)))
        g0 = min(g0 + CHUNK_G, NG)
    TloC = max(sum(T_lo[a:b]) for a, b in chunks)
    ThiC = max(sum(T_hi[a:b]) for a, b in chunks)
    TovC = max(sum(T_ov[a:b]) for a, b in chunks)

    with tile.TileContext(nc, trace_sim=False) as tc:
        with (
            tc.tile_pool(name="singles", bufs=1) as sg,
            tc.tile_pool(name="dram", bufs=1, space="DRAM") as dram,
        ):
            def load(ap_in, shape, dt, name):
                t = sg.tile(list(shape), dt, name=name)
                nc.sync.dma_start(out=t[:], in_=ap_in[:])
                return t

            W1_sb = load(W1, [P, D], F32, "W1_sb")
            b1_sb = load(b1row, [1, D], F32, "b1_sb")
            o1f_sb = load(ones1f, [1, D], F32, "o1f_sb")
            o1b_sb = load(ones1b, [1, D], BF, "o1b_sb")
            g1_sb = load(g1b, [P, D], F32, "g1_sb")
            bt1_sb = load(bt1b, [P, D], F32, "bt1_sb")
            We1T_sb = load(We1T, [P, D], BF, "We1T_sb")
            We1B_sb = load(We1B, [P, D], BF, "We1B_sb")
            be1_sb = load(be1row, [1, D], BF, "be1_sb")
            We2_sb = load(We2c, [P, 1], BF, "We2_sb")
            be2_sb = load(be2c, [P, 1], F32, "be2_sb")
            WgT_sb = load(WgT, [P, D], BF, "WgT_sb")
            WgB_sb = load(WgB, [P, D], BF, "WgB_sb")
            bg_sb = load(bgc, [P, 1], F32, "bg_sb")
            g2_sb = load(g2b, [P, D], F32, "g2_sb")
            bt2_sb = load(bt2b, [P, D], F32, "bt2_sb")
            idn_sb = load(idn, [P, P], BF, "idn_sb")
            iota_sb = load(iota, [P, P], BF, "iota_sb")
            ixlo_sb = load(ixlo, [P, max(NTlo, 1) * 8], I16, "ixlo_sb")
            ixhi_sb = load(ixhi, [P, max(NThi, 1) * 8], I16, "ixhi_sb")
            ixb_sb = load(ixb, [P, max(NTb, 1) * 8], I16, "ixb_sb")
            crel_sb = load(colrel, [P, max(NT, 1)], F32, "crel_sb")

            eps_sb = sg.tile([P, 1], F32, name="eps_sb")
            nc.vector.memset(eps_sb[:], 1e-5)

            hT_sb = sg.tile([P, NLOC], BF, name="hT_sb")
            B_sb = sg.tile([P, NLOC], BF, name="B_sb")
            hn_sb = sg.tile([P, NLOC], BF, name="hn_sb")
            mvall = sg.tile([P, 2 * NG], F32, name="mvall")

            HA_shard = dram.tile([NLOC, 2 * D], BF, name="HA_shard")
            HA_full = dram.tile(
                [NPAD, 2 * D], BF, name="HA_full", addr_space="Shared"
            )
            B_dram = dram.tile([NLOC, D], BF, name="B_dram")

            # ================= phase 1: node transform ==================
            with (
                tc.tile_pool(name="xtp", bufs=1) as xtp,
                tc.tile_pool(name="ps1", bufs=2, space="PSUM") as ps1,
                tc.tile_pool(name="w1p", bufs=3) as w1p,
            ):
                xT_sb = xtp.tile([P, NLOC], F32, name="xT_sb")
                nc.sync.dma_start(out=xT_sb[:], in_=xT[:])
                h_sb = xtp.tile([P, NLOC], BF, name="h_sb")
                A_sb = xtp.tile([P, NLOC], BF, name="A_sb")
                for g in range(NG):
                    gsl = slice(g * P, (g + 1) * P)
                    hp = ps1.tile([P, D], F32, tag="hpre", name=f"hp{g}")
                    nc.tensor.matmul(
                        out=hp[:], lhsT=xT_sb[:, gsl], rhs=W1_sb[:],
                        start=True, stop=False,
                    )
                    nc.tensor.matmul(
                        out=hp[:], lhsT=o1f_sb[:], rhs=b1_sb[:],
                        start=False, stop=True,
                    )
                    st = w1p.tile([P, 6], F32, tag="st", name=f"st{g}")
                    nc.vector.bn_stats(out=st[:], in_=hp[:])
                    mv = w1p.tile([P, 2], F32, tag="mv", name=f"mv{g}")
                    nc.vector.bn_aggr(out=mv[:], in_=st[:])
                    sd = w1p.tile([P, 1], F32, tag="sd", name=f"sd{g}")
                    nc.scalar.activation(
                        out=sd[:], in_=mv[:, 1:2],
                        func=mybir.ActivationFunctionType.Sqrt,
                        bias=eps_sb[:],
                    )
                    rstd = w1p.tile([P, 1], F32, tag="rstd", name=f"rs{g}")
                    nc.vector.reciprocal(out=rstd[:], in_=sd[:])
                    t1 = w1p.tile([P, D], F32, tag="t1", name=f"t1{g}")
                    nc.vector.tensor_scalar(
                        out=t1[:], in0=hp[:], scalar1=mv[:, 0:1],
                        scalar2=rstd[:], op0=mybir.AluOpType.subtract,
                        op1=mybir.AluOpType.mult,
                    )
                    u1 = w1p.tile([P, D], F32, tag="u1", name=f"u1{g}")
                    nc.vector.tensor_mul(out=u1[:], in0=t1[:], in1=g1_sb[:])
                    v1 = w1p.tile([P, D], F32, tag="v1", name=f"v1{g}")
                    nc.vector.tensor_add(out=v1[:], in0=u1[:], in1=bt1_sb[:])
                    nc.vector.tensor_scalar_max(
                        out=h_sb[:, gsl], in0=v1[:], scalar1=0.0
                    )
                    htp = ps1.tile([P, D], F32, tag="hT", name=f"htp{g}")
                    nc.tensor.matmul(
                        out=htp[:], lhsT=h_sb[:, gsl], rhs=idn_sb[:],
                        start=True, stop=True,
                    )
                    nc.any.tensor_copy(out=hT_sb[:, gsl], in_=htp[:])
                    ap_ = ps1.tile([P, D], F32, tag="A", name=f"apz{g}")
                    nc.tensor.matmul(
                        out=ap_[:], lhsT=hT_sb[:, gsl], rhs=We1T_sb[:],
                        start=True, stop=False,
                    )
                    nc.tensor.matmul(
                        out=ap_[:], lhsT=o1b_sb[:], rhs=be1_sb[:],
                        start=False, stop=True,
                    )
                    nc.any.tensor_copy(out=A_sb[:, gsl], in_=ap_[:])
                    bp = ps1.tile([P, D], F32, tag="B", name=f"bp{g}")
                    nc.tensor.matmul(
                        out=bp[:], lhsT=hT_sb[:, gsl], rhs=We1B_sb[:],
                        start=True, stop=True,
                    )
                    nc.any.tensor_copy(out=B_sb[:, gsl], in_=bp[:])

                ha_v = HA_shard.rearrange("(g p) c -> p g c", p=P)
                nc.sync.dma_start(
                    out=ha_v[:, :, 0:D],
                    in_=h_sb.rearrange("p (g j) -> p g j", g=NG),
                )
                nc.sync.dma_start(
                    out=ha_v[:, :, D : 2 * D],
                    in_=A_sb.rearrange("p (g j) -> p g j", g=NG),
                )
                nc.sync.dma_start(
                    out=B_dram.rearrange("(g p) c -> p g c", p=P)[:, :, :],
                    in_=B_sb.rearrange("p (g j) -> p g j", g=NG),
                )
            nc.gpsimd.collective_compute(
                "AllGather",
                mybir.AluOpType.bypass,
                replica_groups=[list(range(N_CORES))],
                ins=[HA_shard.opt()],
                outs=[HA_full.opt()],
            )

            # ================= phase 2+3: edges + update ================
            with (
                tc.tile_pool(name="pz", bufs=2, space="PSUM") as pz,
                tc.tile_pool(name="psc", bufs=2, space="PSUM") as psc,
                tc.tile_pool(name="pag", bufs=2, space="PSUM") as pag,
                tc.tile_pool(name="pg3", bufs=1, space="PSUM") as pg3,
                tc.tile_pool(name="gio", bufs=2) as gio,
                tc.tile_pool(name="wrk", bufs=3) as wrk,
                tc.tile_pool(name="osb", bufs=2) as osb,
            ):
                lo_off = 0
                hi_off = 0
                b_off = 0
                t_off = 0
                qctr = [0]

                def next_q():
                    q = qctr[0] % 4
                    qctr[0] += 1
                    return q

                for (ga, gb) in chunks:
                    nlo = sum(T_lo[ga:gb])
                    nhi = sum(T_hi[ga:gb])
                    nov = sum(T_ov[ga:gb])
                    halo = gio.tile(
                        [P, max(TloC, 1), 2 * D], BF, tag="halo",
                        name=f"halo{ga}",
                    )
                    hahi = gio.tile(
                        [P, max(ThiC, 1), 2 * D], BF, tag="hahi",
                        name=f"hahi{ga}",
                    )
                    btc = gio.tile(
                        [P, max(TovC, 1), D], BF, tag="btc", name=f"btc{ga}"
                    )
                    if nlo:
                        nc.gpsimd.dma_gather(
                            out_ap=halo[:, 0:nlo, :],
                            in_ap=HA_full[0:hi_base, :],
                            idxs_ap=ixlo_sb[:, lo_off * 8 : (lo_off + nlo) * 8],
                            num_idxs=nlo * P,
                            num_idxs_reg=nlo * P,
                            elem_size=2 * D,
                            single_packet=False,
                            queue_num=next_q(),
                        )
                    if nhi:
                        nc.gpsimd.dma_gather(
                            out_ap=hahi[:, 0:nhi, :],
                            in_ap=HA_full[hi_base:, :],
                            idxs_ap=ixhi_sb[:, hi_off * 8 : (hi_off + nhi) * 8],
                            num_idxs=nhi * P,
                            num_idxs_reg=nhi * P,
                            elem_size=2 * D,
                            single_packet=False,
                            queue_num=next_q(),
                        )
                    if nov:
                        nc.gpsimd.dma_gather(
                            out_ap=btc[:, 0:nov, :],
                            in_ap=B_dram[:, :],
                            idxs_ap=ixb_sb[:, b_off * 8 : (b_off + nov) * 8],
                            num_idxs=nov * P,
                            num_idxs_reg=nov * P,
                            elem_size=D,
                            single_packet=False,
                            queue_num=next_q(),
                        )
                    lpos = 0
                    hpos = 0
                    bpos = 0
                    for g in range(ga, gb):
                        Tg = T_all[g]
                        a_lo, a_hi = T_alo[g], T_ahi[g]
                        o_lo = T_ovlo[g]
                        gsl = slice(g * P, (g + 1) * P)

                        def srcs(j, _l=lpos, _h=hpos, _b=bpos, _alo=a_lo,
                                 _ahi=a_hi, _olo=o_lo):
                            """tile j -> (ha_slice, b_is_resident, b_slice)"""
                            if j < _alo:
                                return halo[:, _l + j, :], True, None
                            j2 = j - _alo
                            if j2 < _ahi:
                                return hahi[:, _h + j2, :], True, None
                            j3 = j2 - _ahi
                            if j3 < _olo:
                                return (halo[:, _l + _alo + j3, :], False,
                                        btc[:, _b + j3, :])
                            j4 = j3 - _olo
                            return (hahi[:, _h + _ahi + j4, :], False,
                                    btc[:, _b + _olo + j4, :])

                        s_ps = psc.tile([P, Tg], F32, tag="s", name=f"s{g}")
                        aggr = pag.tile([P, P], F32, tag="aggr", name=f"ag{g}")
                        w_sb = wrk.tile([P, Tg], F32, tag="w", name=f"w{g}")
                        nck = (Tg + SCORE_BATCH - 1) // SCORE_BATCH
                        for c in range(nck):
                            tl = c * SCORE_BATCH
                            th = min(tl + SCORE_BATCH, Tg)
                            z = pz.tile(
                                [P, SCORE_BATCH * P], F32, tag="z",
                                name=f"z{g}_{c}",
                            )
                            for i, t in enumerate(range(tl, th)):
                                zsl = slice(i * P, (i + 1) * P)
                                ha_s, b_res, b_s = srcs(t)
                                nc.tensor.matmul(
                                    out=z[:, zsl], lhsT=ha_s[:, D : 2 * D],
                                    rhs=idn_sb[:], start=True, stop=False,
                                )
                                nc.tensor.matmul(
                                    out=z[:, zsl],
                                    lhsT=(B_sb[:, gsl] if b_res else b_s),
                                    rhs=idn_sb[:], start=False, stop=True,
                                )
                            wl = (th - tl) * P
                            r = wrk.tile(
                                [P, SCORE_BATCH * P], BF, tag="r",
                                name=f"r{g}_{c}",
                            )
                            nc.scalar.activation(
                                out=r[:, 0:wl], in_=z[:, 0:wl],
                                func=mybir.ActivationFunctionType.Relu,
                            )
                            for i, t in enumerate(range(tl, th)):
                                nc.tensor.matmul(
                                    out=s_ps[:, t : t + 1],
                                    lhsT=r[:, i * P : (i + 1) * P],
                                    rhs=We2_sb[:], start=True, stop=True,
                                )
                        nc.scalar.activation(
                            out=w_sb[:], in_=s_ps[:, 0:Tg],
                            func=mybir.ActivationFunctionType.Sigmoid,
                            bias=be2_sb[:],
                        )
                        for t in range(Tg):
                            ha_s, _, _ = srcs(t)
                            m = wrk.tile([P, P], BF, tag="m", name=f"m{g}_{t}")
                            nc.vector.tensor_scalar(
                                out=m[:], in0=iota_sb[:],
                                scalar1=crel_sb[:, t_off + t : t_off + t + 1],
                                scalar2=w_sb[:, t : t + 1],
                                op0=mybir.AluOpType.is_equal,
                                op1=mybir.AluOpType.mult,
                            )
                            nc.tensor.matmul(
                                out=aggr[:], lhsT=ha_s[:, 0:D], rhs=m[:],
                                start=(t == 0), stop=(t == Tg - 1),
                            )
                        # ---- phase 3: gate + blend ----
                        ragg = wrk.tile([P, P], BF, tag="ragg", name=f"rg{g}")
                        nc.any.tensor_copy(out=ragg[:], in_=aggr[:])
                        gp = pg3.tile([P, P], F32, tag="gate", name=f"gp{g}")
                        nc.tensor.matmul(
                            out=gp[:], lhsT=WgT_sb[:], rhs=hT_sb[:, gsl],
                            start=True, stop=False,
                        )
                        nc.tensor.matmul(
                            out=gp[:], lhsT=WgB_sb[:], rhs=ragg[:],
                            start=False, stop=True,
                        )
                        gate = wrk.tile([P, P], BF, tag="gate_sb",
                                        name=f"gt{g}")
                        nc.scalar.activation(
                            out=gate[:], in_=gp[:],
                            func=mybir.ActivationFunctionType.Sigmoid,
                            bias=bg_sb[:],
                        )
                        d1 = wrk.tile([P, P], BF, tag="d1", name=f"d1{g}")
                        nc.vector.tensor_tensor(
                            out=d1[:], in0=ragg[:], in1=hT_sb[:, gsl],
                            op=mybir.AluOpType.subtract,
                        )
                        d2 = wrk.tile([P, P], BF, tag="d2", name=f"d2{g}")
                        nc.vector.tensor_mul(out=d2[:], in0=gate[:], in1=d1[:])
                        nc.vector.tensor_add(
                            out=hn_sb[:, gsl], in0=hT_sb[:, gsl], in1=d2[:]
                        )
                        lpos += T_lo[g]
                        hpos += T_hi[g]
                        bpos += T_ov[g]
                        t_off += Tg
                    lo_off += nlo
                    hi_off += nhi
                    b_off += nov

                # ============== phase 4: final LayerNorm ===============
                for g in range(NG):
                    gsl = slice(g * P, (g + 1) * P)
                    hnp = pg3.tile([P, P], F32, tag="hnp", name=f"hnp{g}")
                    nc.tensor.matmul(
                        out=hnp[:], lhsT=hn_sb[:, gsl], rhs=idn_sb[:],
                        start=True, stop=True,
                    )
                    st3 = wrk.tile([P, 6], F32, tag="st3", name=f"st3{g}")
                    nc.vector.bn_stats(out=st3[:], in_=hnp[:])
                    nc.vector.bn_aggr(
                        out=mvall[:, 2 * g : 2 * g + 2], in_=st3[:]
                    )
                var_v = mvall.rearrange("p (g two) -> p g two", two=2)
                sdall = sg.tile([P, NG], F32, name="sdall")
                nc.scalar.activation(
                    out=sdall[:], in_=var_v[:, :, 1],
                    func=mybir.ActivationFunctionType.Sqrt,
                    bias=eps_sb[:],
                )
                rstdall = sg.tile([P, NG], F32, name="rstdall")
                nc.vector.reciprocal(out=rstdall[:], in_=sdall[:])
                for g in range(NG):
                    gsl = slice(g * P, (g + 1) * P)
                    hnp2 = pg3.tile([P, P], F32, tag="hnp", name=f"hnp2_{g}")
                    nc.tensor.matmul(
                        out=hnp2[:], lhsT=hn_sb[:, gsl], rhs=idn_sb[:],
                        start=True, stop=True,
                    )
                    t1o = osb.tile([P, D], F32, tag="t1o", name=f"t1o{g}")
                    nc.vector.tensor_scalar(
                        out=t1o[:], in0=hnp2[:],
                        scalar1=mvall[:, 2 * g : 2 * g + 1],
                        scalar2=rstdall[:, g : g + 1],
                        op0=mybir.AluOpType.subtract,
                        op1=mybir.AluOpType.mult,
                    )
                    u1o = osb.tile([P, D], F32, tag="u1o", name=f"u1o{g}")
                    nc.vector.tensor_mul(out=u1o[:], in0=t1o[:], in1=g2_sb[:])
                    o1o = osb.tile([P, D], F32, tag="o1o", name=f"o1o{g}")
                    nc.vector.tensor_add(out=o1o[:], in0=u1o[:], in1=bt2_sb[:])
                    nc.sync.dma_start(out=out[gsl, :], in_=o1o[:])

    nc.compile()
    return nc


# ----------------------------------------------------------------------------
# host-side sharding + launch
# ----------------------------------------------------------------------------

_CACHE = {}


def _wrap16(seq):
    """idx i -> [i%16, i//16], replicated to 128 partitions (8 Q7 cores)."""
    n = len(seq)
    if n == 0:
        return np.zeros((P, 8), np.int16)
    assert n % 16 == 0
    blk = np.asarray(seq, np.int16).reshape(-1, 16).T
    return np.tile(blk, (8, 1))


def _pick_region(degs_per_core):
    """degs_per_core: [n_cores, 128].  Pick aligned tile count a minimizing
    a + max_k ceil(overflow_k / 128); return (a, ov_tiles)."""
    dmax = int(degs_per_core.max()) if degs_per_core.size else 0
    if dmax == 0:
        return 0, 0
    best = (1 << 30, 0, 0)
    for a in range(dmax + 1):
        ov = np.maximum(degs_per_core - a, 0).sum(axis=1)
        ovt = int(math.ceil(ov.max() / P)) if ov.max() else 0
        tot = a + ovt
        if tot < best[0]:
            best = (tot, a, ovt)
    return best[1], best[2]


import math  # noqa: E402


def _group_seqs(rg, cg, T_a, T_o, base):
    """Edges (rows rg, group-local cols cg) of one (group, half).
    Returns (row_seq  [ (T_a+T_o)*128 ], colrel_aligned [T_a*128],
    colrel_ov [T_o*128], bcols_ov [T_o*128])."""
    r_h = rg - base
    ordh = np.lexsort((r_h, cg))
    r_h, c_h = r_h[ordh], cg[ordh]
    if len(c_h):
        starts = np.r_[0, np.cumsum(np.bincount(c_h, minlength=P))]
        rank = np.arange(len(c_h)) - starts[c_h]
    else:
        rank = np.zeros(0, np.int64)
    al = rank < T_a
    a_rows = np.zeros((T_a, P), np.int32)
    a_mask = np.zeros((T_a, P), bool)
    a_rows[rank[al], c_h[al]] = r_h[al]
    a_mask[rank[al], c_h[al]] = True
    a_cr = np.where(
        a_mask, np.arange(P, dtype=np.float32)[None, :], np.float32(-1.0)
    )
    r_o = r_h[~al]
    c_o = c_h[~al]
    n_o = len(r_o)
    assert n_o <= T_o * P, (n_o, T_o)
    o_rows = np.zeros(T_o * P, np.int32)
    o_cr = np.full(T_o * P, -1.0, np.float32)
    o_cols = np.zeros(T_o * P, np.int32)
    o_rows[:n_o] = r_o
    o_cr[:n_o] = c_o.astype(np.float32)
    o_cols[:n_o] = c_o
    return (
        np.concatenate([a_rows.reshape(-1), o_rows]),
        a_cr.reshape(-1), o_cr, o_cols,
    )


def kernel(
    x, edge_index, W1, b1, g1, bt1, We1, be1, We2, be2,
    Wn1, bn1, Wn2, bn2, Wg, bg, g2, bt2, _trace=False,
):
    x = np.asarray(x, dtype=np.float32)
    N = x.shape[0]
    NG = (N + N_CORES * P - 1) // (N_CORES * P)
    NLOC = NG * P
    NPAD = NLOC * N_CORES

    row = np.asarray(edge_index[0], dtype=np.int64)
    col = np.asarray(edge_index[1], dtype=np.int64)
    order = np.argsort(col, kind="stable")
    row_s = row[order].astype(np.int32)
    col_s = col[order].astype(np.int32)
    bounds = np.searchsorted(col_s, np.arange(N_CORES + 1) * NLOC)

    deg_lo = np.zeros((N_CORES, NLOC), np.int32)
    deg_hi = np.zeros((N_CORES, NLOC), np.int32)
    for k in range(N_CORES):
        lo, hi = bounds[k], bounds[k + 1]
        cl = col_s[lo:hi] - k * NLOC
        ish = row_s[lo:hi] >= SPLIT
        deg_lo[k] = np.bincount(cl[~ish], minlength=NLOC)
        deg_hi[k] = np.bincount(cl[ish], minlength=NLOC)

    T_alo, T_ahi, T_ovlo, T_ovhi = [], [], [], []
    for g in range(NG):
        csl = slice(g * P, (g + 1) * P)
        a, o = _pick_region(deg_lo[:, csl])
        T_alo.append(a)
        T_ovlo.append(o)
        a, o = _pick_region(deg_hi[:, csl])
        T_ahi.append(a)
        T_ovhi.append(o)
        if T_alo[g] + T_ahi[g] + T_ovlo[g] + T_ovhi[g] == 0:
            T_ovlo[g] = 1
    T_all = [T_alo[g] + T_ahi[g] + T_ovlo[g] + T_ovhi[g] for g in range(NG)]
    NT = sum(T_all)

    key = (N, NG, tuple(T_alo), tuple(T_ahi), tuple(T_ovlo), tuple(T_ovhi))
    if key not in _CACHE:
        _CACHE[key] = _build_program(
            NG, NLOC, NPAD, T_alo, T_ahi, T_ovlo, T_ovhi
        )
    nc = _CACHE[key]

    bf = lambda a: np.ascontiguousarray(np.asarray(a, np.float32)).astype(BF16)
    f32 = lambda a: np.ascontiguousarray(np.asarray(a, np.float32))
    shared = {
        "W1": f32(W1),
        "b1row": f32(b1).reshape(1, D),
        "ones1f": np.ones((1, D), np.float32),
        "ones1b": np.ones((1, D), BF16),
        "g1b": np.broadcast_to(f32(g1).reshape(1, D), (P, D)).copy(),
        "bt1b": np.broadcast_to(f32(bt1).reshape(1, D), (P, D)).copy(),
        "We1T": bf(We1[:D]),
        "We1B": bf(We1[D:]),
        "be1row": bf(be1).reshape(1, D),
        "We2c": bf(We2).reshape(P, 1),
        "be2c": np.broadcast_to(f32(be2).reshape(1, 1), (P, 1)).copy(),
        "WgT": bf(Wg[:D]),
        "WgB": bf(Wg[D:]),
        "bgc": f32(bg).reshape(P, 1),
        "g2b": np.broadcast_to(f32(g2).reshape(1, D), (P, D)).copy(),
        "bt2b": np.broadcast_to(f32(bt2).reshape(1, D), (P, D)).copy(),
        "idn": np.eye(P, dtype=BF16),
        "iota": np.broadcast_to(
            np.arange(P, dtype=np.float32).reshape(1, P), (P, P)
        ).astype(BF16),
    }

    xp = np.zeros((NPAD, D), np.float32)
    xp[:N] = x

    in_maps = []
    for k in range(N_CORES):
        lo, hi = bounds[k], bounds[k + 1]
        rk = row_s[lo:hi]
        ck = col_s[lo:hi] - k * NLOC
        gk = ck // P
        seq_lo, seq_hi, seq_b, seq_cr = [], [], [], []
        for g in range(NG):
            gmask = gk == g
            rg = rk[gmask]
            cg = (ck[gmask] - g * P).astype(np.int64)
            ishg = rg >= SPLIT
            lo_rows, lo_acr, lo_ocr, lo_ocols = _group_seqs(
                rg[~ishg], cg[~ishg], T_alo[g], T_ovlo[g], 0
            )
            hi_rows, hi_acr, hi_ocr, hi_ocols = _group_seqs(
                rg[ishg], cg[ishg], T_ahi[g], T_ovhi[g], SPLIT
            )
            seq_lo.append(lo_rows)
            seq_hi.append(hi_rows)
            seq_b.append(lo_ocols + g * P)
            seq_b.append(hi_ocols + g * P)
            # processing order: alo, ahi, ovlo, ovhi
            seq_cr.extend([lo_acr, hi_acr, lo_ocr, hi_ocr])
        slo = np.concatenate(seq_lo)
        shi = np.concatenate(seq_hi)
        sb = np.concatenate(seq_b)
        scr = np.concatenate(seq_cr)
        assert len(scr) == NT * P, (len(scr), NT * P)
        im = dict(shared)
        im["xT"] = np.ascontiguousarray(xp[k * NLOC : (k + 1) * NLOC].T)
        im["ixlo"] = _wrap16(slo)
        im["ixhi"] = _wrap16(shi)
        im["ixb"] = _wrap16(sb)
        im["colrel"] = np.ascontiguousarray(scr.reshape(NT, P).T)
        in_maps.append(im)

    if _trace:
        _install_ntff_hook()
    res = run_bass_kernel_spmd(
        nc, in_maps, core_ids=list(range(N_CORES)), trace=_trace
    )
    out = np.concatenate(
        [res.results[k]["out"] for k in range(N_CORES)], axis=0
    )[:N]
    if _trace:
        kernel.last_exec_time_ns = res.exec_time_ns
    return np.ascontiguousarray(out, dtype=np.float32)
